# Initial kernel scaffold
#
"""MACE symmetric-contraction (order-3, out={}) kernel for 8 Trainium2 cores.

Problem (hardcoded): N=2048 nodes, C=128 channels, D=9 (0e+1o+2e), S=50
species, chunks [(7,1),(11,3),(12,5)], gradient_normalization 0.5.

    wn   = w_c[index] * (mul**-0.5)**GN                 (N, mul, C)
    out_c[n,c,a,b,i] = sum_{j,k} u_c[a,b,j,k,i] wn[n,k,c] x[n,c,j]
    out  = concat_c(out_c, axis=-1)                     (N, C, 9, 9, 9)

Strategy: data-parallel over nodes (256/core). Host pre-gathers the small
species tables (w_c[index], pure data movement) and pre-transposes x; the
device builds, per node quad, z[(k*9+j), ch] = xT[j,ch]*wg[k,ch] via
TensorE replication matmuls + a VectorE multiply, contracts z against the
(column-padded, fp32r) U bases into PSUM, interleaves the three chunks'
irrep columns into (a,b,i) rows on ScalarE/VectorE, and streams the
(N*C, 729) fp32 result out. The kernel is HBM-write-bound (~95.5 MB/core).
"""

from contextlib import ExitStack

import numpy as np

N_NODES = 2048
N_CORES = 8
C = 128
D = 9
GN = 0.5
CHUNKS = [(7, 1), (11, 3), (12, 5)]  # (mul, ir)
K_OFFS = [0, 7, 18]                  # chunk row offset in the 30-row w block
I_OFFS = [0, 1, 4]                   # output irrep interleave offsets
PS_OFFS = [0, 256, 512]              # psum column offset per chunk
N_PAD = [256, 256, 405]              # padded matmul N per chunk (fp32r >=256)

_CACHE = {}


class _SplitDrainTileContext:
    """Lazy import wrapper; see _make_tc_class."""


def _make_tc_class():
    import concourse.tile as tile
    from concourse.vector_clock import ScopedClock, VectorClock

    class SplitDrainTileContext(tile.TileContext):
        # The walrus build in this container rejects instructions carrying
        # more than one sync wait. Tile's stock exit emits a single Drain
        # waiting on every outstanding semaphore; split it into one
        # single-wait NOP per logical processor instead.
        def _drain_and_barrier(self, tick_clock, wait_clock):
            vc = tick_clock.global_clock
            n = len(vc)
            for p in range(n):
                t = vc[p]
                if t > 0:
                    single = VectorClock([t if i == p else 0 for i in range(n)])
                    nop = self.nc.sync.nop()
                    wait_clock.add_sem_waits(nop.ins, ScopedClock({None: single}))
            self.nc.sync.drain()
            self.nc.all_engine_barrier()
            popped = self.nc._tile_sem_poison_stack.pop()
            assert popped is self._sem_poison
            self.nc.clear_and_free_semaphores(list(self.sems.allocated().values()))
            self.nc.all_engine_barrier()

    return SplitDrainTileContext


def build_nc(n_nodes, repeats=1, xr_via_sbuf=True):
    import concourse.bass as bass
    from concourse import mybir

    F32 = mybir.dt.float32
    F32R = mybir.dt.float32r
    mm_dtype = F32R

    assert n_nodes % 16 == 0
    n_quads = n_nodes // 4
    nc = bass.Bass()

    xt_d = nc.dram_tensor("xt", [n_quads * 32, 512], mm_dtype, kind="ExternalInput")
    wq_d = nc.dram_tensor("wq", [n_quads * 32, 512], mm_dtype, kind="ExternalInput")
    u_d = [
        nc.dram_tensor(f"u{i}p", [9 * CHUNKS[i][0], N_PAD[i]], mm_dtype,
                       kind="ExternalInput")
        for i in range(3)
    ]
    rx_d = nc.dram_tensor("rx", [128, 108], mm_dtype, kind="ExternalInput")
    rw_d = nc.dram_tensor("rw", [128, 324], mm_dtype, kind="ExternalInput")
    out_d = nc.dram_tensor("out", [n_nodes * 128, 729], F32, kind="ExternalOutput")

    with ExitStack() as ctx:
        tc = ctx.enter_context(_make_tc_class()(nc))
        consts = ctx.enter_context(tc.tile_pool(name="consts", bufs=1))
        xt_pool = ctx.enter_context(tc.tile_pool(name="xt", bufs=2))
        w_pool = ctx.enter_context(tc.tile_pool(name="w", bufs=2))
        xrep_ps = ctx.enter_context(tc.tile_pool(name="xrep", bufs=1, space="PSUM"))
        wrep_ps = ctx.enter_context(tc.tile_pool(name="wrep", bufs=3, space="PSUM"))
        outp_ps = ctx.enter_context(tc.tile_pool(name="outp", bufs=2, space="PSUM"))
        zt_pool = ctx.enter_context(tc.tile_pool(name="zt", bufs=2))
        xr_sb = ctx.enter_context(tc.tile_pool(name="xrsb", bufs=2))
        outs_pool = ctx.enter_context(tc.tile_pool(name="outs", bufs=3))

        u_t = []
        for i in range(3):
            t = consts.tile([9 * CHUNKS[i][0], N_PAD[i]], mm_dtype, tag=f"u{i}")
            nc.sync.dma_start(t[:], u_d[i][:])
            u_t.append(t)
        rx_t = consts.tile([128, 108], mm_dtype, tag="rx")
        nc.sync.dma_start(rx_t[:], rx_d[:])
        rw_t = consts.tile([128, 324], mm_dtype, tag="rw")
        nc.sync.dma_start(rw_t[:], rw_d[:])

        for rep in range(repeats):
            xt_t = None
            w_t = None
            for q in range(n_quads):
                if q % 4 == 0:
                    xt_t = xt_pool.tile([128, 512], mm_dtype, tag="xt")
                    nc.sync.dma_start(xt_t[:], xt_d[q * 32 : q * 32 + 128, :])
                    w_t = w_pool.tile([128, 512], mm_dtype, tag="w")
                    nc.sync.dma_start(w_t[:], wq_d[q * 32 : q * 32 + 128, :])
                g = 32 * (q % 4)

                # x-replication: psum[k*9+j, (t,ch)] = xT[j, (t,ch)]
                xr = xrep_ps.tile([108, 512], F32, tag="xr")
                nc.tensor.matmul(
                    xr[:],
                    rx_t[g : g + 9, :],
                    xt_t[g : g + 9, :],
                    start=True, stop=True,
                    tile_position=(g, 0),
                )
                if xr_via_sbuf:
                    xru = xr_sb.tile([108, 512], mm_dtype, tag="xrs")
                    nc.vector.tensor_copy(xru[:], xr[:])
                else:
                    xru = xr

                zts = []
                for ci, (mul, ir) in enumerate(CHUNKS):
                    # w-replication via zero-padded selector (K=30 covers all
                    # chunks; zero lhsT rows mask the other chunks):
                    # psum[k*9+j, (t,ch)] = wg[off_ci+k, (t,ch)]
                    wr = wrep_ps.tile([9 * mul, 512], F32, tag="wr")
                    nc.tensor.matmul(
                        wr[:],
                        rw_t[g : g + 30, 108 * ci : 108 * ci + 9 * mul],
                        w_t[g : g + 30, :],
                        start=True, stop=True,
                        tile_position=(g, 0),
                    )
                    zt = zt_pool.tile([9 * mul, 512], mm_dtype, tag=f"zt{ci}")
                    nc.vector.tensor_mul(zt[:], wr[:], xru[0 : 9 * mul, :])
                    zts.append(zt)

                for t in range(4):
                    node = q * 4 + t
                    op = outp_ps.tile([128, 1024], F32, tag="op")
                    for ci in range(3):
                        nc.tensor.matmul(
                            op[:, PS_OFFS[ci] : PS_OFFS[ci] + N_PAD[ci]],
                            zts[ci][:, 128 * t : 128 * t + 128],
                            u_t[ci][:],
                            start=True, stop=True,
                        )
                    os_ = outs_pool.tile([128, 81, 9], F32, tag="os")
                    # interleave chunk cols into (ab, iglob)
                    nc.vector.tensor_copy(
                        os_[:, :, 0:1],
                        op[:, 0:81].rearrange("p (a i) -> p a i", i=1),
                    )
                    nc.scalar.copy(
                        os_[:, :, 1:4],
                        op[:, 256 : 256 + 243].rearrange("p (a i) -> p a i", i=3),
                    )
                    nc.scalar.copy(
                        os_[:, :, 4:9],
                        op[:, 512 : 512 + 405].rearrange("p (a i) -> p a i", i=5),
                    )
                    nc.sync.dma_start(
                        out_d[node * 128 : node * 128 + 128, :],
                        os_[:],
                    )
    return nc


def make_u_consts(u0, u1, u2):
    consts = {}
    for ci, (u, (mul, ir)) in enumerate(zip((u0, u1, u2), CHUNKS)):
        # U[(k*9+j), (a*9+b)*ir + i] = u[a,b,j,k,i]
        U = np.ascontiguousarray(u.transpose(3, 2, 0, 1, 4)).reshape(mul * 9, 81 * ir)
        Up = np.zeros((mul * 9, N_PAD[ci]), np.float32)
        Up[:, : 81 * ir] = U
        consts[f"u{ci}p"] = Up
    rx = np.zeros((128, 108), np.float32)
    for g in range(4):
        for j in range(9):
            rx[32 * g + j, j::9] = 1.0
    consts["rx"] = rx
    rw = np.zeros((128, 324), np.float32)
    for g in range(4):
        for ci, (mul, ir) in enumerate(CHUNKS):
            for k in range(mul):
                r = 32 * g + K_OFFS[ci] + k
                rw[r, 108 * ci + 9 * k : 108 * ci + 9 * k + 9] = 1.0
    consts["rw"] = rw
    return consts


def make_in_maps(node_feats, index, u0, u1, u2, w0, w1, w2):
    x = np.asarray(node_feats, dtype=np.float32)
    idx = np.asarray(index)
    ws = [np.asarray(w, dtype=np.float32) for w in (w0, w1, w2)]
    us = [np.asarray(u, dtype=np.float32) for u in (u0, u1, u2)]

    # species gather + normalization fold (pure data movement / rescale)
    wcat = np.concatenate(
        [w[idx] * (w.shape[1] ** -0.5) ** GN for w in ws], axis=1
    )  # (N, 30, C)

    consts = make_u_consts(*us)
    per = N_NODES // N_CORES
    n_quads = per // 4
    maps = []
    for c in range(N_CORES):
        xc = x[c * per : (c + 1) * per]
        wc = wcat[c * per : (c + 1) * per]
        xt = np.zeros((n_quads, 32, 512), np.float32)
        xt[:, :9, :] = (
            xc.reshape(n_quads, 4, 128, 9).transpose(0, 3, 1, 2).reshape(n_quads, 9, 512)
        )
        wq = np.zeros((n_quads, 32, 512), np.float32)
        wq[:, :30, :] = (
            wc.reshape(n_quads, 4, 30, 128).transpose(0, 2, 1, 3).reshape(n_quads, 30, 512)
        )
        m = {"xt": xt.reshape(n_quads * 32, 512), "wq": wq.reshape(n_quads * 32, 512)}
        m.update(consts)
        maps.append(m)
    return maps


def get_nc(repeats=1):
    key = ("nc", N_NODES // N_CORES, repeats)
    if key not in _CACHE:
        _CACHE[key] = build_nc(N_NODES // N_CORES, repeats=repeats)
    return _CACHE[key]


def run_device(maps, repeats=1):
    from concourse.bass_utils import run_bass_kernel_spmd

    nc = get_nc(repeats)
    res = run_bass_kernel_spmd(nc, maps, core_ids=list(range(N_CORES)))
    return res


def kernel(node_feats, index, u0, u1, u2, w0, w1, w2):
    maps = make_in_maps(node_feats, index, u0, u1, u2, w0, w1, w2)
    res = run_device(maps)
    per = N_NODES // N_CORES
    out = np.empty((N_NODES, C, D, D, D), np.float32)
    for c in range(N_CORES):
        out[c * per : (c + 1) * per] = res.results[c]["out"].reshape(per, C, D, D, D)
    return out


# revision 1
# speedup vs baseline: 566.1925x; 566.1925x over previous
"""MACE symmetric-contraction (order-3, out={}) kernel for 8 Trainium2 cores.

Problem (hardcoded): N=2048 nodes, C=128 channels, D=9 (0e+1o+2e), S=50
species, chunks [(7,1),(11,3),(12,5)], gradient_normalization 0.5.

    wn   = w_c[index] * (mul**-0.5)**GN                 (N, mul, C)
    out_c[n,c,a,b,i] = sum_{j,k} u_c[a,b,j,k,i] wn[n,k,c] x[n,c,j]
    out  = concat_c(out_c, axis=-1)                     (N, C, 9, 9, 9)

Strategy: data-parallel over nodes (256/core). Host pre-gathers the small
species tables (w_c[index], pure data movement) and pre-transposes x; the
device builds, per node quad, z[(k*9+j), ch] = xT[j,ch]*wg[k,ch] via
TensorE replication matmuls + a VectorE multiply, contracts z against the
(column-padded, fp32r) U bases into PSUM, interleaves the three chunks'
irrep columns into (a,b,i) rows on ScalarE/VectorE, and streams the
(N*C, 729) fp32 result out. The kernel is HBM-write-bound (~95.5 MB/core).
"""

from contextlib import ExitStack

import numpy as np

N_NODES = 2048
N_CORES = 8
C = 128
D = 9
GN = 0.5
CHUNKS = [(7, 1), (11, 3), (12, 5)]  # (mul, ir)
K_OFFS = [0, 7, 18]                  # chunk row offset in the 30-row w block
I_OFFS = [0, 1, 4]                   # output irrep interleave offsets
PS_OFFS = [0, 256, 512]              # psum column offset per chunk
N_PAD = [256, 256, 405]              # padded matmul N per chunk (fp32r >=256)

_CACHE = {}


class _SplitDrainTileContext:
    """Lazy import wrapper; see _make_tc_class."""


def _make_tc_class():
    import concourse.tile as tile
    from concourse.vector_clock import ScopedClock, VectorClock

    class SplitDrainTileContext(tile.TileContext):
        # The walrus build in this container rejects instructions carrying
        # more than one sync wait. Tile's stock exit emits a single Drain
        # waiting on every outstanding semaphore; split it into one
        # single-wait NOP per logical processor instead.
        def _drain_and_barrier(self, tick_clock, wait_clock):
            vc = tick_clock.global_clock
            n = len(vc)
            for p in range(n):
                t = vc[p]
                if t > 0:
                    single = VectorClock([t if i == p else 0 for i in range(n)])
                    nop = self.nc.sync.nop()
                    wait_clock.add_sem_waits(nop.ins, ScopedClock({None: single}))
            self.nc.sync.drain()
            self.nc.all_engine_barrier()
            popped = self.nc._tile_sem_poison_stack.pop()
            assert popped is self._sem_poison
            self.nc.clear_and_free_semaphores(list(self.sems.allocated().values()))
            self.nc.all_engine_barrier()

    return SplitDrainTileContext


def build_nc(n_nodes, repeats=1, xr_via_sbuf=True):
    import concourse.bass as bass
    from concourse import mybir

    F32 = mybir.dt.float32
    F32R = mybir.dt.float32r
    mm_dtype = F32R

    assert n_nodes % 16 == 0
    n_quads = n_nodes // 4
    nc = bass.Bass()

    xt_d = nc.dram_tensor("xt", [n_quads * 32, 512], mm_dtype, kind="ExternalInput")
    wq_d = nc.dram_tensor("wq", [n_quads * 32, 512], mm_dtype, kind="ExternalInput")
    u_d = [
        nc.dram_tensor(f"u{i}p", [9 * CHUNKS[i][0], N_PAD[i]], mm_dtype,
                       kind="ExternalInput")
        for i in range(3)
    ]
    rx_d = nc.dram_tensor("rx", [128, 108], mm_dtype, kind="ExternalInput")
    rw_d = nc.dram_tensor("rw", [128, 324], mm_dtype, kind="ExternalInput")
    out_d = nc.dram_tensor("out", [n_nodes * 128, 729], F32, kind="ExternalOutput")

    with ExitStack() as ctx:
        tc = ctx.enter_context(_make_tc_class()(nc))
        consts = ctx.enter_context(tc.tile_pool(name="consts", bufs=1))
        xt_pool = ctx.enter_context(tc.tile_pool(name="xt", bufs=2))
        w_pool = ctx.enter_context(tc.tile_pool(name="w", bufs=2))
        xrep_ps = ctx.enter_context(tc.tile_pool(name="xrep", bufs=1, space="PSUM"))
        wrep_ps = ctx.enter_context(tc.tile_pool(name="wrep", bufs=3, space="PSUM"))
        outp_ps = ctx.enter_context(tc.tile_pool(name="outp", bufs=2, space="PSUM"))
        zt_pool = ctx.enter_context(tc.tile_pool(name="zt", bufs=2))
        xr_sb = ctx.enter_context(tc.tile_pool(name="xrsb", bufs=2))
        outs_pool = ctx.enter_context(tc.tile_pool(name="outs", bufs=3))

        u_t = []
        for i in range(3):
            t = consts.tile([9 * CHUNKS[i][0], N_PAD[i]], mm_dtype, tag=f"u{i}")
            nc.sync.dma_start(t[:], u_d[i][:])
            u_t.append(t)
        rx_t = consts.tile([128, 108], mm_dtype, tag="rx")
        nc.sync.dma_start(rx_t[:], rx_d[:])
        rw_t = consts.tile([128, 324], mm_dtype, tag="rw")
        nc.sync.dma_start(rw_t[:], rw_d[:])

        for rep in range(repeats):
            xt_t = None
            w_t = None
            for q in range(n_quads):
                if q % 4 == 0:
                    xt_t = xt_pool.tile([128, 512], mm_dtype, tag="xt")
                    nc.sync.dma_start(xt_t[:], xt_d[q * 32 : q * 32 + 128, :])
                    w_t = w_pool.tile([128, 512], mm_dtype, tag="w")
                    nc.sync.dma_start(w_t[:], wq_d[q * 32 : q * 32 + 128, :])
                g = 32 * (q % 4)

                # x-replication: psum[k*9+j, (t,ch)] = xT[j, (t,ch)]
                xr = xrep_ps.tile([108, 512], F32, tag="xr")
                nc.tensor.matmul(
                    xr[:],
                    rx_t[g : g + 9, :],
                    xt_t[g : g + 9, :],
                    start=True, stop=True,
                    tile_position=(g, 0),
                )
                if xr_via_sbuf:
                    xru = xr_sb.tile([108, 512], mm_dtype, tag="xrs")
                    nc.vector.tensor_copy(xru[:], xr[:])
                else:
                    xru = xr

                zts = []
                for ci, (mul, ir) in enumerate(CHUNKS):
                    # w-replication via zero-padded selector (K=30 covers all
                    # chunks; zero lhsT rows mask the other chunks):
                    # psum[k*9+j, (t,ch)] = wg[off_ci+k, (t,ch)]
                    wr = wrep_ps.tile([9 * mul, 512], F32, tag="wr")
                    nc.tensor.matmul(
                        wr[:],
                        rw_t[g : g + 30, 108 * ci : 108 * ci + 9 * mul],
                        w_t[g : g + 30, :],
                        start=True, stop=True,
                        tile_position=(g, 0),
                    )
                    zt = zt_pool.tile([9 * mul, 512], mm_dtype, tag=f"zt{ci}")
                    nc.vector.tensor_mul(zt[:], wr[:], xru[0 : 9 * mul, :])
                    zts.append(zt)

                for t in range(4):
                    node = q * 4 + t
                    op = outp_ps.tile([128, 1024], F32, tag="op")
                    for ci in range(3):
                        nc.tensor.matmul(
                            op[:, PS_OFFS[ci] : PS_OFFS[ci] + N_PAD[ci]],
                            zts[ci][:, 128 * t : 128 * t + 128],
                            u_t[ci][:],
                            start=True, stop=True,
                        )
                    os_ = outs_pool.tile([128, 81, 9], F32, tag="os")
                    # interleave chunk cols into (ab, iglob)
                    nc.vector.tensor_copy(
                        os_[:, :, 0:1],
                        op[:, 0:81].rearrange("p (a i) -> p a i", i=1),
                    )
                    nc.scalar.copy(
                        os_[:, :, 1:4],
                        op[:, 256 : 256 + 243].rearrange("p (a i) -> p a i", i=3),
                    )
                    nc.scalar.copy(
                        os_[:, :, 4:9],
                        op[:, 512 : 512 + 405].rearrange("p (a i) -> p a i", i=5),
                    )
                    nc.sync.dma_start(
                        out_d[node * 128 : node * 128 + 128, :],
                        os_[:],
                    )
    return nc


def make_u_consts(u0, u1, u2):
    consts = {}
    for ci, (u, (mul, ir)) in enumerate(zip((u0, u1, u2), CHUNKS)):
        # U[(k*9+j), (a*9+b)*ir + i] = u[a,b,j,k,i]
        U = np.ascontiguousarray(u.transpose(3, 2, 0, 1, 4)).reshape(mul * 9, 81 * ir)
        Up = np.zeros((mul * 9, N_PAD[ci]), np.float32)
        Up[:, : 81 * ir] = U
        consts[f"u{ci}p"] = Up
    rx = np.zeros((128, 108), np.float32)
    for g in range(4):
        for j in range(9):
            rx[32 * g + j, j::9] = 1.0
    consts["rx"] = rx
    rw = np.zeros((128, 324), np.float32)
    for g in range(4):
        for ci, (mul, ir) in enumerate(CHUNKS):
            for k in range(mul):
                r = 32 * g + K_OFFS[ci] + k
                rw[r, 108 * ci + 9 * k : 108 * ci + 9 * k + 9] = 1.0
    consts["rw"] = rw
    return consts


def make_in_maps(node_feats, index, u0, u1, u2, w0, w1, w2):
    x = np.asarray(node_feats, dtype=np.float32)
    idx = np.asarray(index)
    ws = [np.asarray(w, dtype=np.float32) for w in (w0, w1, w2)]
    us = [np.asarray(u, dtype=np.float32) for u in (u0, u1, u2)]

    # species gather + normalization fold (pure data movement / rescale)
    wcat = np.concatenate(
        [w[idx] * (w.shape[1] ** -0.5) ** GN for w in ws], axis=1
    )  # (N, 30, C)

    consts = make_u_consts(*us)
    per = N_NODES // N_CORES
    n_quads = per // 4
    maps = []
    for c in range(N_CORES):
        xc = x[c * per : (c + 1) * per]
        wc = wcat[c * per : (c + 1) * per]
        xt = np.zeros((n_quads, 32, 512), np.float32)
        xt[:, :9, :] = (
            xc.reshape(n_quads, 4, 128, 9).transpose(0, 3, 1, 2).reshape(n_quads, 9, 512)
        )
        wq = np.zeros((n_quads, 32, 512), np.float32)
        wq[:, :30, :] = (
            wc.reshape(n_quads, 4, 30, 128).transpose(0, 2, 1, 3).reshape(n_quads, 30, 512)
        )
        m = {"xt": xt.reshape(n_quads * 32, 512), "wq": wq.reshape(n_quads * 32, 512)}
        m.update(consts)
        maps.append(m)
    return maps


def get_nc(repeats=1):
    key = ("nc", N_NODES // N_CORES, repeats)
    if key not in _CACHE:
        _CACHE[key] = build_nc(N_NODES // N_CORES, repeats=repeats)
    return _CACHE[key]


def run_device(maps, repeats=1):
    from concourse.bass_utils import run_bass_kernel_spmd

    nc = get_nc(repeats)
    res = run_bass_kernel_spmd(nc, maps, core_ids=list(range(N_CORES)))
    return res


def kernel(node_feats, index, u0, u1, u2, w0, w1, w2):
    maps = make_in_maps(node_feats, index, u0, u1, u2, w0, w1, w2)
    res = run_device(maps)
    per = N_NODES // N_CORES
    out = np.empty((N_NODES, C, D, D, D), np.float32)
    for c in range(N_CORES):
        out[c * per : (c + 1) * per] = res.results[c]["out"].reshape(per, C, D, D, D)
    return out
